# revision 29
# baseline (speedup 1.0000x reference)
"""Trainium2 Bass kernel for AttentionModelBase decode step (B=512, N=1000, D=128, H=8).

Strategy:
  - Pure data parallel: 64 batches per core x 8 cores.
  - Host stages embeddings per-core in TWO fp16 layouts:
      embT: [d=128 partitions, n free]   (for d-contractions: scores, logits)
      embN: [n-tile=128 partitions, d free] (for n-contraction: attn @ emb)
    so no on-chip transposes of embeddings are needed.
  - Per-batch query vectors w_q = (W_K,h @ q_h)/sqrt(dh) are host-precomputed
    and staged BLOCK-DIAGONALLY: 16 accumulating full-width matmuls produce the
    whole group's scores stacked [16b*8h, 1024] in PSUM (PE outputs must start
    at partition 0/32/64/96, so per-batch 8-row outputs can't be packed
    directly).  Same trick for logits with an on-chip-built diagonal.
  - Masked exp with row sums from ACT accum_out (scores ~ +-2, no max-sub
    needed); normalize attn with a per-partition scalar; PE-transpose attn;
    accumulate e_att over n-tiles; small fp32 chain (heads/glimpse/logit
    query); logits; tanh clip; mask; log-softmax (again no max-sub: logits
    within +-10).
  - N padded 1000 -> 1024; padded columns are masked infeasible and sliced off.
"""

import math
import os
import sys

for _p in ("/opt/trn_rl_repo", "/root/.axon_site/_ro/trn_rl_repo"):
    if os.path.isdir(_p) and _p not in sys.path:
        sys.path.insert(0, _p)

import numpy as np

import concourse.bacc as bacc
import concourse.bass as bass
import concourse.tile as tile
from concourse import mybir

B, N, D, H = 512, 1000, 128, 8
NP = 1024          # padded N
NCORES = 8
BC = B // NCORES   # 64 batches per core
G = 4              # batch groups per core
GB = BC // G       # 16 batches per group
NT = NP // 128     # 8 n-tiles per batch
F16 = mybir.dt.float16
F32 = mybir.dt.float32
NEG = -30000.0     # additive mask value (exp underflows to exactly 0)
TANH_CLIP = 10.0
AF = mybir.ActivationFunctionType


def build_program(reps=1):
    nc = bacc.Bacc("TRN2")

    embT = nc.dram_tensor("embT", [G, 128, GB * NP], F16, kind="ExternalInput")
    embN = nc.dram_tensor("embN", [G, 128, GB * NT * 128], F16, kind="ExternalInput")
    wqd = nc.dram_tensor("wqd", [G, 128, GB * 128], F16, kind="ExternalInput")
    wvo = nc.dram_tensor("wvo", [128, H * 128], F16, kind="ExternalInput")
    wlts = nc.dram_tensor("wlts", [128, 128], F32, kind="ExternalInput")
    id16 = nc.dram_tensor("id16", [128, 128], F16, kind="ExternalInput")
    id32 = nc.dram_tensor("id32", [128, 128], F32, kind="ExternalInput")
    maskn = nc.dram_tensor("maskn", [G, 16, NP], F16, kind="ExternalInput")
    indic16 = nc.dram_tensor("indic16", [16, 128], F16, kind="ExternalInput")
    mask01 = nc.dram_tensor("mask01", [GB, G * NP], mybir.dt.int8, kind="ExternalInput")
    neginf1 = nc.dram_tensor("neginf1", [GB, NP], F32, kind="ExternalInput")
    out = nc.dram_tensor("log_p", [BC, NP], F32, kind="ExternalOutput")

    with tile.TileContext(nc) as tc:
        with (
            tc.tile_pool(name="const", bufs=1) as const,
            tc.tile_pool(name="big", bufs=7) as big,
            tc.tile_pool(name="mid", bufs=2) as mid,
            tc.tile_pool(name="fin", bufs=2) as fin,
            tc.tile_pool(name="ps_sc", bufs=2, space="PSUM") as ps_sc,
            tc.tile_pool(name="ps_ea", bufs=2, space="PSUM") as ps_ea,
            tc.tile_pool(name="ps_sm", bufs=1, space="PSUM") as ps_sm,
            tc.tile_pool(name="ps_lg", bufs=2, space="PSUM") as ps_lg,
        ):
            # ---- constants (loaded once) ----
            wvo_sb = const.tile([128, H * 128], F16)
            nc.scalar.dma_start(out=wvo_sb, in_=wvo[:, :])
            wlts_sb = const.tile([128, 128], F32)
            nc.scalar.dma_start(out=wlts_sb, in_=wlts[:, :])
            id16_sb = const.tile([128, 128], F16)
            nc.scalar.dma_start(out=id16_sb, in_=id16[:, :])
            id32_sb = const.tile([128, 128], F32)
            nc.scalar.dma_start(out=id32_sb, in_=id32[:, :])
            ind_sb = const.tile([16, 128], F16)
            nc.scalar.dma_start(out=ind_sb, in_=indic16[:, :])
            ninf_sb = const.tile([GB, NP], F32)
            nc.scalar.dma_start(out=ninf_sb, in_=neginf1[:, :])
            wqd_all = const.tile([128, G * GB * 128], F16)
            for g in range(G):
                eng = nc.sync if g == 0 else nc.scalar
                eng.dma_start(out=wqd_all[:, g * GB * 128:(g + 1) * GB * 128],
                              in_=wqd[g, :, :])
            maskn_all = const.tile([16, G * NP], F16)
            mk = maskn[:, :, :]
            mk_v = bass.AP(tensor=mk.tensor, offset=mk.offset,
                           ap=[[NP, 16], [16 * NP, G], [1, NP]])
            nc.scalar.dma_start(out=maskn_all, in_=mk_v)
            neg3_sb = const.tile([GB, NP], F32)
            nc.vector.memset(neg3_sb, NEG)
            m01_sb = const.tile([GB, G * NP], mybir.dt.int8)
            nc.scalar.dma_start(out=m01_sb, in_=mask01[:, :])

            for rep in range(reps):
              for g in range(G):
                # ---- load group embeddings in quarters (4 batches each) ----
                eTq = []
                eNq = []
                for q in range(4):
                    t_ = big.tile([128, 4 * NP], F16, tag="eT", name=f"eT{q}")
                    nc.sync.dma_start(
                        out=t_, in_=embT[g, :, q * 4 * NP:(q + 1) * 4 * NP])
                    eTq.append(t_)
                for hq in range(2):
                    t2_ = big.tile([128, 8 * NT * 128], F16, tag="eN",
                                   name=f"eN{hq}", bufs=3)
                    nc.sync.dma_start(
                        out=t2_,
                        in_=embN[g, :, hq * 8 * NT * 128:(hq + 1) * 8 * NT * 128])
                    eNq.append(t2_)

                # ---- scores halves: block-diag accumulation + mask matmul ----
                attn_sb = mid.tile([128, NP], F16, tag="attn")
                sums_h = [mid.tile([128, 1], F32, tag=f"sums{s}", name=f"sums{s}") for s in range(2)]
                for s in range(2):
                    sc_ps = ps_sc.tile([128, 512], F32, tag="scores")
                    for b in range(GB):
                        nc.tensor.matmul(
                            sc_ps,
                            lhsT=wqd_all[:, (g * GB + b) * 128:(g * GB + b + 1) * 128],
                            rhs=eTq[b // 4][:, (b % 4) * NP + s * 512:
                                            (b % 4) * NP + (s + 1) * 512],
                            start=(b == 0), stop=False,
                        )
                    nc.tensor.matmul(
                        sc_ps,
                        lhsT=ind_sb,
                        rhs=maskn_all[:, g * NP + s * 512: g * NP + (s + 1) * 512],
                        start=False, stop=True,
                    )
                    nc.scalar.activation(attn_sb[:, s * 512:(s + 1) * 512], sc_ps,
                                         AF.Exp, accum_out=sums_h[s])

                # ---- normalize attn ----
                sums = mid.tile([128, 1], F32, tag="sums")
                nc.vector.tensor_add(sums, sums_h[0], sums_h[1])
                recip = mid.tile([128, 1], F32, tag="recip")
                nc.vector.reciprocal(recip, sums)
                nc.vector.tensor_scalar_mul(attn_sb, attn_sb, recip)

                # ---- transpose attn -> [n, (t, 16b*8h)] ----
                attnT_sb = mid.tile([128, NT * 128], F16, tag="attnT")
                tp = ps_sm.tile([128, NT * 128], F16, tag="smh")
                for t in range(NT):
                    nc.tensor.transpose(tp[:, t * 128:(t + 1) * 128],
                                        attn_sb[:, t * 128:(t + 1) * 128],
                                        id16_sb)
                nc.vector.tensor_copy(attnT_sb, tp)

                # ---- e_att accumulation over n-tiles ----
                eatt_sb = mid.tile([8, GB * 128], F32, tag="eatt_sb")
                for bq in range(GB // 4):
                    ea_ps = ps_ea.tile([8, 4 * 128], F32, tag="ea")
                    for bo in range(4):
                        b = bq * 4 + bo
                        for t in range(NT):
                            nc.tensor.matmul(
                                ea_ps[:, bo * 128:(bo + 1) * 128],
                                lhsT=attnT_sb[:, t * 128 + b * 8:
                                              t * 128 + (b + 1) * 8],
                                rhs=eNq[b // 8][:, ((b % 8) * NT + t) * 128:
                                                ((b % 8) * NT + t + 1) * 128],
                                start=(t == 0), stop=(t == NT - 1),
                            )
                    nc.scalar.copy(eatt_sb[:, bq * 512:(bq + 1) * 512], ea_ps)

                # ---- transpose e_att -> eattT [d, (16b, 8h)] ----
                eaT_ps = ps_sm.tile([128, GB * 8], F32, tag="sm")
                for b in range(GB):
                    nc.tensor.transpose(eaT_ps[:, b * 8:(b + 1) * 8],
                                        eatt_sb[:, b * 128:(b + 1) * 128],
                                        id32_sb[0:8, 0:8])
                eattT_sb = mid.tile([128, GB * 8], F16, tag="eattT")
                nc.vector.tensor_copy(eattT_sb, eaT_ps)

                # ---- glimpse^T = sum_h Wvo_h^T @ eattT_h  (heads folded in) ----
                glimT_ps = ps_sm.tile([128, GB], F32, tag="sm")
                ea_r = eattT_sb.rearrange("p (b h) -> p h b", h=H)
                for h in range(H):
                    nc.tensor.matmul(
                        glimT_ps,
                        lhsT=wvo_sb[:, h * 128:(h + 1) * 128],
                        rhs=ea_r[:, h, :],
                        start=(h == 0), stop=(h == H - 1),
                    )
                glimT_sb = mid.tile([128, GB], F32, tag="glimT")
                nc.vector.tensor_copy(glimT_sb, glimT_ps)

                # ---- gl = (W_logit^T glimpse)/sqrt(D), scattered to diag ----
                glT_ps = ps_sm.tile([128, GB], F32, tag="sm")
                nc.tensor.matmul(glT_ps, lhsT=wlts_sb, rhs=glimT_sb,
                                 start=True, stop=True)
                gld = mid.tile([128, GB * GB], F16, tag="gld")
                nc.vector.memset(gld, 0.0)
                gld_diag = gld.rearrange("p (b c) -> p b c", c=GB)
                diag_view = bass.AP(tensor=gld_diag.tensor, offset=gld_diag.offset,
                                    ap=[gld_diag.ap[0], [GB + 1, GB]])
                nc.vector.tensor_copy(diag_view, glT_ps)

                # ---- logits halves + tanh/mask/log-softmax ----
                lm = fin.tile([GB, NP], F32, tag="lm")
                sums2_h = [fin.tile([GB, 1], F32, tag=f"s2_{s}", name=f"s2_{s}") for s in range(2)]
                logp = fin.tile([GB, NP], F32, tag="logp")
                exp2 = logp
                for s in range(2):
                    lg_ps = ps_lg.tile([GB, 512], F32, tag="logits")
                    for b in range(GB):
                        nc.tensor.matmul(
                            lg_ps,
                            lhsT=gld[:, b * GB:(b + 1) * GB],
                            rhs=eTq[b // 4][:, (b % 4) * NP + s * 512:
                                            (b % 4) * NP + (s + 1) * 512],
                            start=(b == 0), stop=(b == GB - 1),
                        )
                    half = slice(s * 512, (s + 1) * 512)
                    nc.scalar.activation(lm[:, half], lg_ps, AF.Tanh)
                    nc.vector.tensor_scalar_mul(lm[:, half], lm[:, half],
                                                TANH_CLIP)
                    nc.vector.copy_predicated(
                        lm[:, half],
                        m01_sb[:, g * NP + s * 512: g * NP + (s + 1) * 512],
                        neg3_sb[:, half])
                    nc.scalar.activation(exp2[:, half], lm[:, half], AF.Exp,
                                         accum_out=sums2_h[s])
                sums2 = fin.tile([GB, 1], F32, tag="sums2")
                nc.vector.tensor_add(sums2, sums2_h[0], sums2_h[1])
                lse = fin.tile([GB, 1], F32, tag="lse")
                nc.scalar.activation(lse, sums2, AF.Ln)
                for s in range(2):
                    half = slice(s * 512, (s + 1) * 512)
                    nc.vector.tensor_scalar_sub(logp[:, half], lm[:, half], lse)
                    nc.vector.copy_predicated(
                        logp[:, half],
                        m01_sb[:, g * NP + s * 512: g * NP + (s + 1) * 512],
                        ninf_sb[:, half])
                    nc.sync.dma_start(
                        out=out[g * GB:(g + 1) * GB, s * 512:(s + 1) * 512],
                        in_=logp[:, half])

    nc.finalize()
    return nc


def stage_core(emb_c, mask_c, W_node, W_fixed, W_step, W_out, W_ph):
    """Host-side staging for one core's 64 batches.

    emb_c: [BC, N, D] f32; mask_c: [BC, N] int32 (1 = infeasible).
    """
    dh = D // H
    embp = np.zeros((BC, NP, D), np.float32)
    embp[:, :N, :] = emb_c
    maskp = np.ones((BC, NP), np.float32)
    maskp[:, :N] = mask_c.astype(np.float32)

    # per-batch query -> w_q, staged block-diagonally per group
    ge = emb_c.mean(axis=1, dtype=np.float64).astype(np.float32)  # [BC, D]
    query = ge @ W_fixed + W_ph @ W_step                          # [BC, D]
    qh = query.reshape(BC, H, dh)
    Wk3 = W_node[:, :D].reshape(D, H, dh)
    wq_all = np.einsum("dhe,bhe->bdh", Wk3, qh) / math.sqrt(dh)   # [BC, D, H]
    wqd = np.zeros((G, GB, D, GB, H), np.float32)
    for b in range(GB):
        wqd[:, b, :, b, :] = wq_all.reshape(G, GB, D, H)[:, b]
    wqd = wqd.reshape(G, GB, D, GB * H).transpose(0, 2, 1, 3)     # [G, D, GB, 128]
    wqd = np.ascontiguousarray(wqd.reshape(G, D, GB * 128)).astype(np.float16)

    e4 = embp.reshape(G, GB, NT, 128, D)
    embT = np.ascontiguousarray(
        e4.transpose(0, 4, 1, 2, 3).reshape(G, 128, GB * NP)
    ).astype(np.float16)
    embN = np.ascontiguousarray(
        e4.transpose(0, 3, 1, 2, 4).reshape(G, 128, GB * NT * 128)
    ).astype(np.float16)

    maskn_st = (NEG * maskp).reshape(G, GB, NP).astype(np.float16)
    ind = np.zeros((GB, GB, H), np.float16)
    for b in range(GB):
        ind[b, b, :] = 1.0
    ind = ind.reshape(GB, 128)

    Wv3 = W_node[:, D:2 * D].reshape(D, H, 16)
    Wo3 = W_out.reshape(H, 16, D)
    wvo = np.einsum("dhj,hjm->dhm", Wv3, Wo3)
    wvo = np.ascontiguousarray(wvo.reshape(D, H * D)).astype(np.float16)

    return {
        "embT": embT,
        "embN": embN,
        "wqd": wqd,
        "wvo": wvo,
        "wlts": np.ascontiguousarray(W_node[:, 2 * D:].T / math.sqrt(D)).astype(
            np.float32),
        "id16": np.eye(128, dtype=np.float16),
        "id32": np.eye(128, dtype=np.float32),
        "maskn": maskn_st,
        "indic16": np.ascontiguousarray(ind),
                "mask01": np.ascontiguousarray(maskp.reshape(G, GB, NP).transpose(1, 0, 2).reshape(GB, G * NP)).astype(np.int8),
        "neginf1": np.full((GB, NP), -np.inf, np.float32),
    }


def stage_all(embeddings, mask, W_node, W_fixed, W_step, W_out, W_ph):
    embeddings = np.asarray(embeddings, np.float32)
    mask2 = np.asarray(mask, np.int32).reshape(B, N)
    args = [np.asarray(a, np.float32) for a in
            (W_node, W_fixed, W_step, W_out, W_ph)]
    in_maps = []
    for c in range(NCORES):
        sl = slice(c * BC, (c + 1) * BC)
        in_maps.append(stage_core(embeddings[sl], mask2[sl], *args))
    return in_maps


def kernel(embeddings, mask, W_node, W_fixed, W_step, W_out, W_ph):
    from concourse.bass_utils import run_bass_kernel_spmd

    in_maps = stage_all(embeddings, mask, W_node, W_fixed, W_step, W_out, W_ph)
    nc = build_program()
    res = run_bass_kernel_spmd(nc, in_maps, core_ids=list(range(NCORES)))
    outs = [res.results[c]["log_p"] for c in range(NCORES)]
    full = np.concatenate(outs, axis=0)          # [B, NP]
    return np.ascontiguousarray(full[:, :N]).reshape(B, 1, N).astype(np.float32)


if __name__ == "__main__":
    nc = build_program()
    print("program built ok")
